# revision 23
# baseline (speedup 1.0000x reference)
"""AttentionBlock3D (GroupNorm -> qkv 1x1 conv -> MHA -> proj -> residual) on 8 trn2 cores.

Sharding: data-parallel over B (2) x query-block (4): core c handles batch c//4 and
queries [(c%4)*1024, (c%4+1)*1024). Keys/values are full-length per core, so there is
no cross-core communication. The S axis of x is rotated per core so every core runs an
identical static program on "queries = first 1024 columns" (softmax and groupnorm are
permutation-invariant along S).

On-device layout (per core, S=4096, C=256, 8 heads, hd=32):
  h = groupnorm(x)                 [256, 4096]   (ch on partitions, 2 chunks)
  q = Wq h (+bq)                   [128, 1024] per head-group g (4 heads x 32)
  k = Wk h (+bk)                   [128, 4096] per head-group
  vT = h^T Wv^T                    [128 k-chunk, 32 chunks x 8 heads x 36]
                                   (v-bias folds into the proj bias on host:
                                   proj(o/S + bv) = proj(o/S) + Wp@bv)
                                   (32 v cols + ones col + pad; computed in
                                   chunks woven into the first attention loop)
  per head-pair p (2 heads), q-window w (512), k-tile t (128 keys):
    logitsT[k,q] = k_tile^T q      2 heads row-packed (tile_position=(32h,0))
                                   into one of two alternating [128,1024] PSUM
                                   tiles so QK[t+1] overlaps exp[t]
    expT = exp(scale*logitsT)      one ACT op per t, PSUM->SBUF
    acc[h] += [vT|1]^T expT        M=33: rows 0-31 = o, row 32 = sum-exp
  o / sumexp via reciprocal + DRAM-bounce partition-broadcast of 1/S,
  proj matmul (+bias via K=1 ones matmul), +residual.
QK runs in fp8e4 (e4m3) with perf_mode=DoubleRow: q and k are quantized
on the DVE (bias-add writes fp8 directly) and DMA-shuffled via a DRAM
bounce into the paired layout [K/2 partitions, 2, cols] the mode needs;
two contraction rows per PE cell double the QK throughput, dropping PE
per-iteration work below the ACT exp stream's 817 ns so the exp pipeline
runs gaplessly (fp8 q/k costs ~2e-3 rel err, well inside the 2e-2 gate).
All other matmuls stay float32r (full PE rate at N>=256); softmax skips
the max-subtraction (|logits*scale| < 8 by construction).

Measured (marginal device time per body, R-repeat differencing): the loop
period is set by per-iteration cross-engine handoffs (PE QK -> ACT exp ->
PE AV with ~0.3-0.6us semaphore wake latency on the consumer) plus PE
work. With fp32r QK the PE per-iteration work (~1.1us) exceeded the ACT
exp stream (~0.82us), so the PE could never run ahead and every exp paid
the wake latency (~470-500us total). fp8 DoubleRow QK halves the QK
stream, dropping PE below ACT: interleaved A/B measured ~250-400us vs
~430-500us for fp32r. Dead ends (all within noise or worse): bf16
operands alone, deeper logit pipelines, exp lagging, software-pipelined
AV, precomputed k/v, batched 2048-col exps; merging the two heads'
matmuls is impossible (a matmul output cannot span a PSUM bank, N <= 512
fp32).
"""

import numpy as np

import concourse.bacc as bacc
import concourse.bass as bass
import concourse.mybir as mybir
import concourse.tile as tile
from concourse.bass_utils import run_bass_kernel_spmd

FP32 = mybir.dt.float32
FP32R = mybir.dt.float32r
BF16 = mybir.dt.bfloat16
FP8 = mybir.dt.float8e4

C = 256
NH = 8
HD = C // NH  # 32
G = 32
EPS = 1e-6
S = 4096
QBLK = 1024  # queries per core
ATT_SCALE = float(HD) ** -0.5
HDP = 36  # per-(chunk,head) vT stride: 16B-aligned (36*4=144), cols = 32 v + 1 ones + 3 pad
NCORES = 8


def build_nc(dbg=False, repeats=1):
    nc = bacc.Bacc("TRN2", debug=False, enable_asserts=False, num_devices=NCORES)

    x_d = nc.dram_tensor("x", [C, S], FP32, kind="ExternalInput").ap()
    wqkvT_d = nc.dram_tensor("wqkvT", [C, 3 * C], FP32, kind="ExternalInput").ap()
    qkb_d = nc.dram_tensor("qkb", [128, 4], FP32, kind="ExternalInput").ap()
    wprojT_d = nc.dram_tensor("wprojT", [C, C], FP32, kind="ExternalInput").ap()
    pb_d = nc.dram_tensor("pb_row", [1, C], FP32, kind="ExternalInput").ap()
    gamma_d = nc.dram_tensor("gamma", [C, 1], FP32, kind="ExternalInput").ap()
    beta_d = nc.dram_tensor("beta", [C, 1], FP32, kind="ExternalInput").ap()
    gmat_d = nc.dram_tensor("gmat", [128, 64], FP32, kind="ExternalInput").ap()
    out_d = nc.dram_tensor("out", [C, QBLK], FP32, kind="ExternalOutput").ap()

    dbg_d = None
    if dbg:
        dbg_d = {nm: nc.dram_tensor(f"dbg_{nm}", shp, FP32,
                                    kind="ExternalOutput").ap()
                 for nm, shp in [("h0", [128, S]), ("h1", [128, S]),
                                 ("q0", [128, QBLK]), ("k0", [128, S]),
                                 ("vT", [128, 32 * NH * HDP]), ("o00", [128, QBLK]),
                                 ("o01", [128, QBLK]), ("recipS", [NH, QBLK]),
                                 ("rep0", [128, QBLK]), ("o0n0", [128, QBLK])]}
    with tile.TileContext(nc) as tc:
        for _ in range(repeats):
            build_body(nc, tc, x_d, wqkvT_d, qkb_d, wprojT_d, pb_d,
                       gamma_d, beta_d, gmat_d, out_d, dbg_d)
    nc.compile()
    return nc


def build_body(nc, tc, x_d, wqkvT_d, qkb_d, wprojT_d, pb_d,
               gamma_d, beta_d, gmat_d, out_d, dbg_d=None):
    import contextlib
    ctx = contextlib.ExitStack()
    with ctx:
        persist = ctx.enter_context(tc.tile_pool(name="persist", bufs=1))

        # ---- load weights / constants ----
        wqkvT = [persist.tile([128, 3 * C], FP32R, name=f"wqkvT{c}", tag=f"wqkvT{c}") for c in range(2)]
        for c in range(2):
            nc.sync.dma_start(out=wqkvT[c], in_=wqkvT_d[128 * c:128 * (c + 1), :].bitcast(FP32R))
        wprojT = [persist.tile([128, C], FP32R, name=f"wprojT{c}", tag=f"wprojT{c}") for c in range(2)]
        for c in range(2):
            nc.sync.dma_start(out=wprojT[c], in_=wprojT_d[128 * c:128 * (c + 1), :].bitcast(FP32R))
        qkb = persist.tile([128, 4], FP32, name="qkb", tag="qkb")
        nc.sync.dma_start(out=qkb, in_=qkb_d)
        pb_row = persist.tile([1, C], FP32R, name="pb", tag="pb")
        nc.sync.dma_start(out=pb_row, in_=pb_d.bitcast(FP32R))
        gamma = [persist.tile([128, 1], FP32, name=f"gamma{c}", tag=f"gamma{c}") for c in range(2)]
        beta = [persist.tile([128, 1], FP32, name=f"beta{c}", tag=f"beta{c}") for c in range(2)]
        for c in range(2):
            nc.sync.dma_start(out=gamma[c], in_=gamma_d[128 * c:128 * (c + 1), :])
            nc.sync.dma_start(out=beta[c], in_=beta_d[128 * c:128 * (c + 1), :])
        gmat = persist.tile([128, 64], FP32, name="gmat", tag="gmat")
        nc.sync.dma_start(out=gmat, in_=gmat_d)
        # memset cannot target fp32r; memset fp32 then round via DVE copy
        ones_f = persist.tile([128, QBLK], FP32, name="ones_f", tag="ones_f")
        nc.vector.memset(ones_f, 1.0)
        ones_q = persist.tile([1, QBLK], FP32R, name="ones_q", tag="ones_q")
        nc.vector.tensor_copy(out=ones_q, in_=ones_f[0:1, :])
        eps_t = persist.tile([32, 1], FP32, name="eps", tag="eps")
        nc.vector.memset(eps_t, EPS)
        # dummy exp: pulls the ACT table load off the critical path (loads
        # the natural_log_exp set while the x DMA is still streaming)
        warm = persist.tile([32, 1], FP32, name="warm", tag="warm")
        nc.scalar.activation(out=warm, in_=eps_t,
                             func=mybir.ActivationFunctionType.Exp, scale=1.0)

        # persistent activation tensors
        h_sb = [persist.tile([128, S], FP32R, name=f"h{c}", tag=f"h{c}") for c in range(2)]
        # q/k in fp8e4, staged in the natural [head*32+row, col] layout then
        # DMA-shuffled into the DoubleRow pairing [head*32 + row//2, row%2,
        # col] (16 used partitions per 32-strip, bases stay 32-aligned for
        # tile_position). The (ki,ko) pairing order is irrelevant as long as
        # q and k use the same shuffle: the contraction sums over all pairs.
        q_sb = [persist.tile([128, QBLK], FP8, name=f"q{g}", tag=f"q{g}") for g in range(2)]
        k_sb = [persist.tile([128, S], FP8, name=f"k{g}", tag=f"k{g}") for g in range(2)]
        q8 = [persist.tile([128, 2, QBLK], FP8, name=f"q8{g}", tag=f"q8{g}")
              for g in range(2)]
        k8 = [persist.tile([128, 2, S], FP8, name=f"k8{g}", tag=f"k8{g}")
              for g in range(2)]

        def dr_shuffle(dst, src_t, cols, col0):
            # dst[32h + j, ko, col0+c] = src[32h + 2j + ko, col0+c]
            # 2D DMAs only: 3D forms get dim-merged by AP balancing and the
            # merged free span overflows the partition row
            for hl in range(4):
                for ko in range(2):
                    sap = src_t[32 * hl + ko:32 * hl + ko + 31,
                                col0:col0 + cols]
                    s2 = bass.AP(tensor=sap.tensor, offset=sap.offset,
                                 ap=[[2 * sap.ap[0][0], 16], sap.ap[1]])
                    d2 = dst[32 * hl:32 * hl + 16, ko, col0:col0 + cols]
                    nc.sync.dma_start(out=d2, in_=s2)
        vT = persist.tile([128, 32, NH, HDP], FP32R, name="vT", tag="vT")
        o0 = [persist.tile([128, QBLK], FP32, name=f"o0{c}", tag=f"o0{c}") for c in range(2)]
        o0n = [persist.tile([128, QBLK], FP32R, name=f"o0n{c}", tag=f"o0n{c}")
               for c in range(2)]
        reps = [persist.tile([128, QBLK], FP32, name=f"rep{c}", tag=f"rep{c}")
                for c in range(2)]
        dram = ctx.enter_context(tc.tile_pool(name="dram", bufs=1, space="DRAM"))
        recipS_d = dram.tile([NH, QBLK], FP32, name="recipS_d", tag="recipS_d")

        # helper convs, emitted either streamed into the groupnorm apply or
        # woven into the attention loop so the PE computes them under the
        # ACT exp stream
        def k_chunk(g, j, pool):
            k_ps = pool.tile([128, 512], FP32, name="k_ps", tag="k_ps", bufs=1)
            for c in range(2):
                nc.tensor.matmul(
                    out=k_ps,
                    lhsT=wqkvT[c][:, C + 128 * g:C + 128 * (g + 1)],
                    rhs=h_sb[c][:, 512 * j:512 * (j + 1)],
                    start=(c == 0), stop=(c == 1))
            nc.vector.tensor_scalar(out=k_sb[g][:, 512 * j:512 * (j + 1)],
                                    in0=k_ps,
                                    scalar1=qkb[:, 2 + g:3 + g], scalar2=None,
                                    op0=mybir.AluOpType.add)
            dr_shuffle(k8[g], k_sb[g], 512, 512 * j)

        def v_chunk(t, pool):
            v_ps = pool.tile([128, NH, HD], FP32, name="v_ps", tag="v_ps", bufs=1)
            v_ps_f = v_ps.rearrange("p a b -> p (a b)")
            for c in range(2):
                nc.tensor.matmul(
                    out=v_ps_f,
                    lhsT=h_sb[c][:, 128 * t:128 * (t + 1)],
                    rhs=wqkvT[c][:, 2 * C:3 * C],
                    start=(c == 0), stop=(c == 1))
            nc.vector.tensor_copy(out=vT[:, t, :, 0:HD], in_=v_ps)

        # ===== P1: GroupNorm (streamed) + start of qkv projections ========
        with tc.tile_pool(name="gn", bufs=1) as gn_pool, \
             tc.tile_pool(name="psum_small", bufs=1, space="PSUM") as psum_small:
            x_sb = [gn_pool.tile([128, S], FP32, name=f"x{c}", tag=f"x{c}") for c in range(2)]
            # slab-wise DMA so bn_stats can start before the full tile lands;
            # alternate two DMA queues so the 16 slabs stream in parallel
            for c in range(2):
                for i in range(8):
                    q = nc.sync if (i % 2 == 0) else nc.gpsimd
                    q.dma_start(
                        out=x_sb[c][:, 512 * i:512 * (i + 1)],
                        in_=x_d[128 * c:128 * (c + 1), 512 * i:512 * (i + 1)])

            # per-channel stats via bn_stats/bn_aggr (free dim), then 8-channel
            # group combine via a tiny matmul against the group-indicator matrix.
            msq = [gn_pool.tile([128, 3], FP32, name=f"msq{c}", tag=f"msq{c}") for c in range(2)]
            gstat_ps = psum_small.tile([32, 3], FP32, name="gstat_ps", tag="gstat_ps")
            for c in range(2):
                xv = x_sb[c].rearrange("p (a b) -> p a b", b=512)
                stats = gn_pool.tile([128, 8, 6], FP32, name=f"stats{c}", tag=f"stats{c}")
                for i in range(8):
                    nc.vector.bn_stats(out=stats[:, i, :], in_=xv[:, i, :])
                mv = gn_pool.tile([128, 2], FP32, name=f"mv{c}", tag=f"mv{c}")
                nc.vector.bn_aggr(out=mv, in_=stats)
                # msq = [mean, var, mean^2]
                nc.vector.tensor_copy(out=msq[c][:, 0:2], in_=mv)
                nc.vector.tensor_mul(out=msq[c][:, 2:3], in0=mv[:, 0:1], in1=mv[:, 0:1])
            for c in range(2):
                nc.tensor.matmul(out=gstat_ps, lhsT=gmat[:, 32 * c:32 * (c + 1)],
                                 rhs=msq[c], start=(c == 0), stop=(c == 1))
            # grp_mean = s0, grp_var = s1 + s2 - s0^2
            gstat = gn_pool.tile([32, 3], FP32, name="gstat", tag="gstat")
            nc.vector.tensor_copy(out=gstat, in_=gstat_ps)
            gvar = gn_pool.tile([32, 1], FP32, name="gvar", tag="gvar")
            gm2 = gn_pool.tile([32, 1], FP32, name="gm2", tag="gm2")
            nc.vector.tensor_mul(out=gm2, in0=gstat[:, 0:1], in1=gstat[:, 0:1])
            nc.vector.tensor_add(out=gvar, in0=gstat[:, 1:2], in1=gstat[:, 2:3])
            nc.vector.tensor_tensor(out=gvar, in0=gvar, in1=gm2,
                                    op=mybir.AluOpType.subtract)
            # rstd = 1/sqrt(var+eps) = exp(-0.5*ln(var+eps)); Ln+Exp share
            # one ACT table set with the attention exps (no table switch)
            nc.scalar.activation(out=gvar, in_=gvar,
                                 func=mybir.ActivationFunctionType.Ln,
                                 bias=eps_t, scale=1.0)
            nc.scalar.activation(out=gvar, in_=gvar,
                                 func=mybir.ActivationFunctionType.Exp,
                                 bias=0.0, scale=-0.5)

            # one broadcast DMA per chunk for [mean, rstd], then per-channel
            # affine A = rstd*gamma, B = beta - mean*A
            mr = gn_pool.tile([32, 2], FP32, name="mr", tag="mr")
            nc.vector.tensor_copy(out=mr[:, 0:1], in_=gstat[:, 0:1])
            nc.vector.tensor_copy(out=mr[:, 1:2], in_=gvar)
            Ab, Bb = [], []
            for c in range(2):
                mr_rep = gn_pool.tile([128, 2], FP32, name=f"mr_rep{c}", tag=f"mr_rep{c}")
                src = mr[16 * c:16 * (c + 1), :]
                bc = bass.AP(tensor=src.tensor, offset=src.offset,
                             ap=[src.ap[0], [0, 8], src.ap[1]])
                nc.sync.dma_start(out=mr_rep, in_=bc)
                A = gn_pool.tile([128, 1], FP32, name=f"A{c}", tag=f"A{c}")
                Bt = gn_pool.tile([128, 1], FP32, name=f"B{c}", tag=f"B{c}")
                nc.vector.tensor_mul(out=A, in0=mr_rep[:, 1:2], in1=gamma[c])
                nc.vector.tensor_mul(out=Bt, in0=mr_rep[:, 0:1], in1=A)
                nc.vector.tensor_tensor(out=Bt, in0=beta[c], in1=Bt,
                                        op=mybir.AluOpType.subtract)
                Ab.append(A)
                Bb.append(Bt)

            # apply h in 512-col slabs; q and the first k chunks stream in as
            # soon as their h columns exist, so the PE starts while the DVE
            # is still applying later slabs
            with tc.tile_pool(name="p2psum", bufs=1, space="PSUM") as p2:
                for j in range(8):
                    for c in range(2):
                        sl = slice(512 * j, 512 * (j + 1))
                        nc.vector.tensor_scalar(out=h_sb[c][:, sl],
                                                in0=x_sb[c][:, sl],
                                                scalar1=Ab[c], scalar2=Bb[c],
                                                op0=mybir.AluOpType.mult,
                                                op1=mybir.AluOpType.add)
                    if j <= 1:
                        k_chunk(0, j, p2)
                    if j == 1:
                        for g in range(2):
                            q_ps = p2.tile([128, QBLK], FP32, name="q_ps",
                                           tag="q_ps", bufs=1)
                            for n in range(2):
                                for c in range(2):
                                    nc.tensor.matmul(
                                        out=q_ps[:, 512 * n:512 * (n + 1)],
                                        lhsT=wqkvT[c][:, 128 * g:128 * (g + 1)],
                                        rhs=h_sb[c][:, 512 * n:512 * (n + 1)],
                                        start=(c == 0), stop=(c == 1))
                            nc.vector.tensor_scalar(out=q_sb[g], in0=q_ps,
                                                    scalar1=qkb[:, g:g + 1],
                                                    scalar2=None,
                                                    op0=mybir.AluOpType.add)
                            dr_shuffle(q8[g], q_sb[g], QBLK, 0)
                nc.vector.tensor_copy(
                    out=vT[:, :, :, HD:HD + 1],
                    in_=ones_f[:, 0:256].rearrange("p (a b c) -> p a b c", b=NH, c=1))

        # ================= P3: attention (head pairs) =====================
        with tc.tile_pool(name="att_psum", bufs=1, space="PSUM") as att_ps, \
             tc.tile_pool(name="acc_psum", bufs=1, space="PSUM") as acc_ps, \
             tc.tile_pool(name="kv_psum", bufs=1, space="PSUM") as kv_ps, \
             tc.tile_pool(name="exp_pool", bufs=6) as exp_pool, \
             tc.tile_pool(name="sums_pool", bufs=3) as sums_pool:
            for p in range(4):
                g, lh = p // 2, 2 * (p % 2)
                for w in range(2):
                    acc = [acc_ps.tile([128, 512], FP32, name=f"acc{hh}",
                                       tag=f"acc{hh}") for hh in range(2)]
                    # double-buffered half-size logits: QK[t+1] overlaps exp[t]
                    lg = [att_ps.tile([128, 1024], FP32, name=f"lg{i}",
                                      tag=f"lg{i}") for i in range(2)]
                    for t in range(32):
                        if p == 0 and w == 0:
                            v_chunk(t, kv_ps)
                            if t % 4 == 0 and t // 4 + 2 <= 7:
                                k_chunk(0, t // 4 + 2, kv_ps)
                        if p == 1 and t % 8 == 0:
                            k_chunk(1, 4 * w + t // 8, kv_ps)
                        buf = lg[t % 2]
                        for hh in range(2):
                            h = lh + hh
                            nc.tensor.matmul(
                                out=buf[:, 512 * hh:512 * (hh + 1)],
                                lhsT=k8[g][32 * h:32 * h + 16, :, 128 * t:128 * (t + 1)],
                                rhs=q8[g][32 * h:32 * h + 16, :, 512 * w:512 * (w + 1)],
                                start=True, stop=True,
                                perf_mode=mybir.MatmulPerfMode.DoubleRow,
                                tile_position=(32 * h, 0))
                        expT = exp_pool.tile([128, 1024], FP32R, name="expT",
                                             tag="expT")
                        nc.scalar.activation(out=expT, in_=buf,
                                             func=mybir.ActivationFunctionType.Exp,
                                             scale=ATT_SCALE)
                        for hh in range(2):
                            nc.tensor.matmul(
                                out=acc[hh][0:HD + 1, :],
                                lhsT=vT[:, t, 4 * g + lh + hh, 0:HD + 1],
                                rhs=expT[:, 512 * hh:512 * (hh + 1)],
                                start=(t == 0), stop=(t == 31),
                                tile_position=(0, 0))
                    for hh in range(2):
                        h = lh + hh
                        # DVE lanes cannot shift partitions; stage at base 0
                        # then move partitions with an SBUF->SBUF DMA.
                        st = sums_pool.tile([33, 512], FP32, name=f"st{hh}",
                                            tag=f"st{hh}")
                        nc.vector.tensor_copy(out=st, in_=acc[hh][0:HD + 1, :])
                        nc.sync.dma_start(
                            out=o0[g][32 * h:32 * (h + 1), 512 * w:512 * (w + 1)],
                            in_=st[0:HD, :])
                        # reciprocal in place (partition 32 is a legal base),
                        # then straight to DRAM for the later broadcast.
                        nc.vector.reciprocal(out=st[HD:HD + 1, :],
                                             in_=st[HD:HD + 1, :])
                        nc.gpsimd.dma_start(
                            out=recipS_d[4 * g + h, 512 * w:512 * (w + 1)],
                            in_=st[HD:HD + 1, :])
                if p % 2 == 1 and w == 1:
                    # this head-group's o0 is complete: normalize it now so
                    # only the projection remains after the last exp
                    src_r = recipS_d[4 * g:4 * (g + 1), :]
                    bc = bass.AP(tensor=src_r.tensor, offset=src_r.offset,
                                 ap=[src_r.ap[0], [0, 32], src_r.ap[1]])
                    nc.sync.dma_start(out=reps[g], in_=bc)
                    nc.vector.tensor_mul(out=o0n[g], in0=o0[g], in1=reps[g])

        # ================= P4: proj + bias + residual =====================
        with tc.tile_pool(name="norm", bufs=1) as norm_pool:
            xres = [norm_pool.tile([128, QBLK], FP32, name=f"xres{m}", tag=f"xres{m}") for m in range(2)]
            for m in range(2):
                nc.sync.dma_start(out=xres[m], in_=x_d[128 * m:128 * (m + 1), 0:QBLK])
            with tc.tile_pool(name="p4psum", bufs=2, space="PSUM") as p4:
                for m in range(2):
                    o_ps = p4.tile([128, QBLK], FP32, name="o_ps", tag="o_ps")
                    for n in range(2):
                        sl = slice(512 * n, 512 * (n + 1))
                        for c in range(2):
                            nc.tensor.matmul(
                                out=o_ps[:, sl],
                                lhsT=wprojT[c][:, 128 * m:128 * (m + 1)],
                                rhs=o0n[c][:, sl],
                                start=(c == 0), stop=False)
                        nc.tensor.matmul(
                            out=o_ps[:, sl],
                            lhsT=pb_row[:, 128 * m:128 * (m + 1)],
                            rhs=ones_q[:, sl],
                            start=False, stop=True)
                    out_sb = norm_pool.tile([128, QBLK], FP32, name=f"outsb{m}", tag=f"outsb{m}")
                    nc.vector.tensor_add(out=out_sb, in0=o_ps, in1=xres[m])
                    nc.sync.dma_start(out=out_d[128 * m:128 * (m + 1), :], in_=out_sb)

            if dbg_d is not None:
                f32 = lambda ap: ap.bitcast(FP32)
                nc.sync.dma_start(out=dbg_d["h0"], in_=f32(h_sb[0]))
                nc.sync.dma_start(out=dbg_d["h1"], in_=f32(h_sb[1]))
                nc.sync.dma_start(out=dbg_d["q0"], in_=f32(q_sb[0]))
                nc.sync.dma_start(out=dbg_d["k0"], in_=f32(k_sb[0]))
                nc.sync.dma_start(out=dbg_d["vT"],
                                  in_=f32(vT.rearrange("p a b c -> p (a b c)")))
                nc.sync.dma_start(out=dbg_d["o00"], in_=o0[0])
                nc.sync.dma_start(out=dbg_d["o01"], in_=o0[1])
                nc.sync.dma_start(out=dbg_d["recipS"], in_=recipS_d)
                nc.sync.dma_start(out=dbg_d["rep0"], in_=reps[0])
                nc.sync.dma_start(out=dbg_d["o0n0"], in_=f32(o0n[0]))


def _host_inputs(x, gn_gamma, gn_beta, qkv_w, qkv_b, proj_w, proj_b):
    B_, C_, D, H, W = x.shape
    S_ = D * H * W
    assert (C_, S_) == (C, S) and B_ == 2
    xf = np.ascontiguousarray(np.asarray(x, np.float32).reshape(B_, C_, S_))
    wqkvT = np.ascontiguousarray(np.asarray(qkv_w, np.float32).T)
    qkb = np.ascontiguousarray(np.asarray(qkv_b[:512], np.float32).reshape(4, 128).T)
    wprojT = np.ascontiguousarray(np.asarray(proj_w, np.float32).T)
    # v-bias folds into the projection bias: proj(o/S + vb) = proj(o/S) + Wp@vb
    pb_eff = np.asarray(proj_b, np.float64) + \
        np.asarray(proj_w, np.float64) @ np.asarray(qkv_b[512:], np.float64)
    pb_row = np.ascontiguousarray(pb_eff.astype(np.float32).reshape(1, C))
    gamma = np.ascontiguousarray(np.asarray(gn_gamma, np.float32).reshape(C, 1))
    beta = np.ascontiguousarray(np.asarray(gn_beta, np.float32).reshape(C, 1))
    gmat = np.zeros((128, 64), np.float32)
    for c in range(2):
        for p in range(128):
            gmat[p, 32 * c + (128 * c + p) // 8] = 0.125
    in_maps = []
    for core in range(NCORES):
        b, qb = core // 4, core % 4
        off = qb * QBLK
        xrot = np.concatenate([xf[b][:, off:], xf[b][:, :off]], axis=1)
        in_maps.append(dict(
            x=np.ascontiguousarray(xrot), wqkvT=wqkvT, qkb=qkb,
            wprojT=wprojT, pb_row=pb_row, gamma=gamma, beta=beta, gmat=gmat))
    return in_maps


_NC_CACHE = None
_EXEC_CACHE = None


class _Exec:
    """Cached PJRT executable for a compiled Bass module.

    run_bass_kernel_spmd (axon path -> run_bass_via_pjrt) builds a fresh
    jax.jit closure per call, re-tracing and re-compiling the XLA wrapper
    every time. Build the sharded executable once and reuse it; inputs are
    re-uploaded only when their bytes change (crc32+adler32 fingerprint).
    """

    def __init__(self, nc, n_cores):
        import jax
        from jax.sharding import Mesh, NamedSharding, PartitionSpec
        from jax.experimental.shard_map import shard_map
        from concourse.bass2jax import (_bass_exec_p, install_neuronx_cc_hook,
                                        partition_id_tensor)
        install_neuronx_cc_hook()
        self.jax = jax
        self.n_cores = n_cores
        pname = nc.partition_id_tensor.name if nc.partition_id_tensor else None
        in_names, out_names, out_avals, zero_outs = [], [], [], []
        for alloc in nc.m.functions[0].allocations:
            if not isinstance(alloc, mybir.MemoryLocationSet):
                continue
            name = alloc.memorylocations[0].name
            if alloc.kind == "ExternalInput":
                if name != pname:
                    in_names.append(name)
            elif alloc.kind == "ExternalOutput":
                out_names.append(name)
                shape = tuple(alloc.tensor_shape)
                dtype = mybir.dt.np(alloc.dtype)
                out_avals.append(jax.core.ShapedArray(shape, dtype))
                zero_outs.append(np.zeros(shape, dtype))
        self.in_names, self.out_names = in_names, out_names
        self.out_avals = out_avals
        all_in = list(in_names) + list(out_names) + ([pname] if pname else [])

        def _body(*args):
            operands = list(args)
            if pname is not None:
                operands.append(partition_id_tensor())
            return tuple(_bass_exec_p.bind(
                *operands, out_avals=tuple(out_avals), in_names=tuple(all_in),
                out_names=tuple(out_names), lowering_input_output_aliases=(),
                sim_require_finite=True, sim_require_nnan=True, nc=nc))

        devices = jax.devices()[:n_cores]
        mesh = Mesh(np.asarray(devices), ("core",))
        specs = (PartitionSpec("core"),) * (len(in_names) + len(out_names))
        self.fn = jax.jit(
            shard_map(_body, mesh=mesh, in_specs=specs,
                      out_specs=(PartitionSpec("core"),) * len(out_names),
                      check_rep=False), keep_unused=True)
        self.sh = NamedSharding(mesh, PartitionSpec("core"))
        self.dev_zero = [jax.device_put(
            np.zeros((n_cores * z.shape[0], *z.shape[1:]), z.dtype), self.sh)
            for z in zero_outs]
        self._in_cache = {}

    def device_inputs(self, in_maps):
        import zlib
        dev_in = []
        for nm in self.in_names:
            glob = np.ascontiguousarray(np.concatenate(
                [np.asarray(in_maps[c][nm]) for c in range(self.n_cores)], 0))
            mv = memoryview(glob).cast("B")
            fp = (zlib.crc32(mv), zlib.adler32(mv), glob.shape, glob.dtype.str)
            hit = self._in_cache.get(nm)
            if hit is None or hit[0] != fp:
                hit = (fp, self.jax.device_put(glob, self.sh))
                self._in_cache[nm] = hit
            dev_in.append(hit[1])
        return dev_in

    def run(self, in_maps):
        outs = self.fn(*self.device_inputs(in_maps), *self.dev_zero)
        outs = [np.asarray(o).reshape(self.n_cores, *self.out_avals[i].shape)
                for i, o in enumerate(outs)]
        return [{nm: outs[i][c] for i, nm in enumerate(self.out_names)}
                for c in range(self.n_cores)]


def kernel(x, gn_gamma, gn_beta, qkv_w, qkv_b, proj_w, proj_b):
    global _NC_CACHE, _EXEC_CACHE
    in_maps = _host_inputs(x, gn_gamma, gn_beta, qkv_w, qkv_b, proj_w, proj_b)
    if _NC_CACHE is None:
        _NC_CACHE = build_nc()
        res = run_bass_kernel_spmd(_NC_CACHE, in_maps,
                                   core_ids=list(range(NCORES)))
        results = res.results
    else:
        if _EXEC_CACHE is None:
            _EXEC_CACHE = _Exec(_NC_CACHE, NCORES)
        results = _EXEC_CACHE.run(in_maps)
    B_, C_, D, H, W = x.shape
    full = np.empty((B_, C, S), np.float32)
    for core in range(NCORES):
        b, qb = core // 4, core % 4
        full[b][:, qb * QBLK:(qb + 1) * QBLK] = results[core]["out"]
    return full.reshape(B_, C, D, H, W)



# revision 24
# speedup vs baseline: 1.1974x; 1.1974x over previous
"""AttentionBlock3D (GroupNorm -> qkv 1x1 conv -> MHA -> proj -> residual) on 8 trn2 cores.

Sharding: data-parallel over B (2) x query-block (4): core c handles batch c//4 and
queries [(c%4)*1024, (c%4+1)*1024). Keys/values are full-length per core, so there is
no cross-core communication. The S axis of x is rotated per core so every core runs an
identical static program on "queries = first 1024 columns" (softmax and groupnorm are
permutation-invariant along S).

On-device layout (per core, S=4096, C=256, 8 heads, hd=32):
  h = groupnorm(x)                 [256, 4096]   (ch on partitions, 2 chunks)
  q = Wq h (+bq)                   [128, 1024] per head-group g (4 heads x 32)
  k = Wk h (+bk)                   [128, 4096] per head-group
  vT = h^T Wv^T                    [128 k-chunk, 32 chunks x 8 heads x 36]
                                   (v-bias folds into the proj bias on host:
                                   proj(o/S + bv) = proj(o/S) + Wp@bv)
                                   (32 v cols + ones col + pad; computed in
                                   chunks woven into the first attention loop)
  per head-pair p (2 heads), q-window w (512), k-tile t (128 keys):
    logitsT[k,q] = k_tile^T q      2 heads row-packed (tile_position=(32h,0))
                                   into one of two alternating [128,1024] PSUM
                                   tiles so QK[t+1] overlaps exp[t]
    expT = exp(scale*logitsT)      one ACT op per t, PSUM->SBUF
    acc[h] += [vT|1]^T expT        M=33: rows 0-31 = o, row 32 = sum-exp
  o / sumexp via reciprocal + DRAM-bounce partition-broadcast of 1/S,
  proj matmul (+bias via K=1 ones matmul), +residual.
QK runs in fp8e4 (e4m3) with perf_mode=DoubleRow: q and k are quantized
on the DVE (bias-add writes fp8 directly) and DMA-shuffled via a DRAM
bounce into the paired layout [K/2 partitions, 2, cols] the mode needs;
two contraction rows per PE cell double the QK throughput, dropping PE
per-iteration work below the ACT exp stream's 817 ns so the exp pipeline
runs gaplessly (fp8 q/k costs ~2e-3 rel err, well inside the 2e-2 gate).
All other matmuls stay float32r (full PE rate at N>=256); softmax skips
the max-subtraction (|logits*scale| < 8 by construction).

Measured (marginal device time per body, R-repeat differencing): the loop
period is set by per-iteration cross-engine handoffs (PE QK -> ACT exp ->
PE AV with ~0.3-0.6us semaphore wake latency on the consumer) plus PE
work. With fp32r QK the PE per-iteration work (~1.1us) exceeded the ACT
exp stream (~0.82us), so the PE could never run ahead and every exp paid
the wake latency (~470-500us total). fp8 DoubleRow QK halves the QK
stream, dropping PE below ACT: interleaved A/B measured ~250-400us vs
~430-500us for fp32r. Dead ends (all within noise or worse): bf16
operands alone, deeper logit pipelines, exp lagging, software-pipelined
AV, precomputed k/v, batched 2048-col exps; merging the two heads'
matmuls is impossible (a matmul output cannot span a PSUM bank, N <= 512
fp32).
"""

import numpy as np

import concourse.bacc as bacc
import concourse.bass as bass
import concourse.mybir as mybir
import concourse.tile as tile
from concourse.bass_utils import run_bass_kernel_spmd

FP32 = mybir.dt.float32
FP32R = mybir.dt.float32r
BF16 = mybir.dt.bfloat16
FP8 = mybir.dt.float8e4

C = 256
NH = 8
HD = C // NH  # 32
G = 32
EPS = 1e-6
S = 4096
QBLK = 1024  # queries per core
ATT_SCALE = float(HD) ** -0.5
HDP = 36  # per-(chunk,head) vT stride: 16B-aligned (36*4=144), cols = 32 v + 1 ones + 3 pad
NCORES = 8


def build_nc(dbg=False, repeats=1):
    nc = bacc.Bacc("TRN2", debug=False, enable_asserts=False, num_devices=NCORES)

    x_d = nc.dram_tensor("x", [C, S], FP32, kind="ExternalInput").ap()
    wqkvT_d = nc.dram_tensor("wqkvT", [C, 3 * C], FP32, kind="ExternalInput").ap()
    qkb_d = nc.dram_tensor("qkb", [128, 4], FP32, kind="ExternalInput").ap()
    wprojT_d = nc.dram_tensor("wprojT", [C, C], FP32, kind="ExternalInput").ap()
    pb_d = nc.dram_tensor("pb_row", [1, C], FP32, kind="ExternalInput").ap()
    gamma_d = nc.dram_tensor("gamma", [C, 1], FP32, kind="ExternalInput").ap()
    beta_d = nc.dram_tensor("beta", [C, 1], FP32, kind="ExternalInput").ap()
    gmat_d = nc.dram_tensor("gmat", [128, 64], FP32, kind="ExternalInput").ap()
    out_d = nc.dram_tensor("out", [C, QBLK], FP32, kind="ExternalOutput").ap()

    dbg_d = None
    if dbg:
        dbg_d = {nm: nc.dram_tensor(f"dbg_{nm}", shp, FP32,
                                    kind="ExternalOutput").ap()
                 for nm, shp in [("h0", [128, S]), ("h1", [128, S]),
                                 ("q0", [128, QBLK]), ("k0", [128, S]),
                                 ("vT", [128, 32 * NH * HDP]), ("o00", [128, QBLK]),
                                 ("o01", [128, QBLK]), ("recipS", [NH, QBLK]),
                                 ("rep0", [128, QBLK]), ("o0n0", [128, QBLK])]}
    with tile.TileContext(nc) as tc:
        for _ in range(repeats):
            build_body(nc, tc, x_d, wqkvT_d, qkb_d, wprojT_d, pb_d,
                       gamma_d, beta_d, gmat_d, out_d, dbg_d)
    nc.compile()
    return nc


def build_body(nc, tc, x_d, wqkvT_d, qkb_d, wprojT_d, pb_d,
               gamma_d, beta_d, gmat_d, out_d, dbg_d=None):
    import contextlib
    ctx = contextlib.ExitStack()
    with ctx:
        persist = ctx.enter_context(tc.tile_pool(name="persist", bufs=1))

        # ---- load weights / constants ----
        wqkvT = [persist.tile([128, 3 * C], FP32R, name=f"wqkvT{c}", tag=f"wqkvT{c}") for c in range(2)]
        for c in range(2):
            nc.sync.dma_start(out=wqkvT[c], in_=wqkvT_d[128 * c:128 * (c + 1), :].bitcast(FP32R))
        wprojT = [persist.tile([128, C], FP32R, name=f"wprojT{c}", tag=f"wprojT{c}") for c in range(2)]
        for c in range(2):
            nc.sync.dma_start(out=wprojT[c], in_=wprojT_d[128 * c:128 * (c + 1), :].bitcast(FP32R))
        qkb = persist.tile([128, 4], FP32, name="qkb", tag="qkb")
        nc.sync.dma_start(out=qkb, in_=qkb_d)
        pb_row = persist.tile([1, C], FP32R, name="pb", tag="pb")
        nc.sync.dma_start(out=pb_row, in_=pb_d.bitcast(FP32R))
        gamma = [persist.tile([128, 1], FP32, name=f"gamma{c}", tag=f"gamma{c}") for c in range(2)]
        beta = [persist.tile([128, 1], FP32, name=f"beta{c}", tag=f"beta{c}") for c in range(2)]
        for c in range(2):
            nc.sync.dma_start(out=gamma[c], in_=gamma_d[128 * c:128 * (c + 1), :])
            nc.sync.dma_start(out=beta[c], in_=beta_d[128 * c:128 * (c + 1), :])
        gmat = persist.tile([128, 64], FP32, name="gmat", tag="gmat")
        nc.sync.dma_start(out=gmat, in_=gmat_d)
        # memset cannot target fp32r; memset fp32 then round via DVE copy
        ones_f = persist.tile([128, QBLK], FP32, name="ones_f", tag="ones_f")
        nc.vector.memset(ones_f, 1.0)
        ones_q = persist.tile([1, QBLK], FP32R, name="ones_q", tag="ones_q")
        nc.vector.tensor_copy(out=ones_q, in_=ones_f[0:1, :])
        eps_t = persist.tile([32, 1], FP32, name="eps", tag="eps")
        nc.vector.memset(eps_t, EPS)
        # dummy exp: pulls the ACT table load off the critical path (loads
        # the natural_log_exp set while the x DMA is still streaming)
        warm = persist.tile([32, 1], FP32, name="warm", tag="warm")
        nc.scalar.activation(out=warm, in_=eps_t,
                             func=mybir.ActivationFunctionType.Exp, scale=1.0)

        # persistent activation tensors
        h_sb = [persist.tile([128, S], FP32R, name=f"h{c}", tag=f"h{c}") for c in range(2)]
        # q/k in fp8e4, staged in the natural [head*32+row, col] layout then
        # DMA-shuffled into the DoubleRow pairing [head*32 + row//2, row%2,
        # col] (16 used partitions per 32-strip, bases stay 32-aligned for
        # tile_position). The (ki,ko) pairing order is irrelevant as long as
        # q and k use the same shuffle: the contraction sums over all pairs.
        q_sb = [persist.tile([128, QBLK], FP8, name=f"q{g}", tag=f"q{g}") for g in range(2)]
        k_sb = [persist.tile([128, S], FP8, name=f"k{g}", tag=f"k{g}") for g in range(2)]
        q8 = [persist.tile([128, 2, QBLK], FP8, name=f"q8{g}", tag=f"q8{g}")
              for g in range(2)]
        k8 = [persist.tile([128, 2, S], FP8, name=f"k8{g}", tag=f"k8{g}")
              for g in range(2)]

        def dr_shuffle(dst, src_t, cols, col0):
            # dst[32h + j, ko, col0+c] = src[32h + 2j + ko, col0+c]
            # 2D DMAs only: 3D forms get dim-merged by AP balancing and the
            # merged free span overflows the partition row
            for hl in range(4):
                for ko in range(2):
                    sap = src_t[32 * hl + ko:32 * hl + ko + 31,
                                col0:col0 + cols]
                    s2 = bass.AP(tensor=sap.tensor, offset=sap.offset,
                                 ap=[[2 * sap.ap[0][0], 16], sap.ap[1]])
                    d2 = dst[32 * hl:32 * hl + 16, ko, col0:col0 + cols]
                    nc.sync.dma_start(out=d2, in_=s2)
        vT = persist.tile([128, 32, NH, HDP], BF16, name="vT", tag="vT")
        o0 = [persist.tile([128, QBLK], FP32, name=f"o0{c}", tag=f"o0{c}") for c in range(2)]
        o0n = [persist.tile([128, QBLK], FP32R, name=f"o0n{c}", tag=f"o0n{c}")
               for c in range(2)]
        reps = [persist.tile([128, QBLK], FP32, name=f"rep{c}", tag=f"rep{c}")
                for c in range(2)]
        dram = ctx.enter_context(tc.tile_pool(name="dram", bufs=1, space="DRAM"))
        recipS_d = dram.tile([NH, QBLK], FP32, name="recipS_d", tag="recipS_d")

        # helper convs, emitted either streamed into the groupnorm apply or
        # woven into the attention loop so the PE computes them under the
        # ACT exp stream
        def k_chunk(g, j, pool):
            k_ps = pool.tile([128, 512], FP32, name="kv_ps", tag="kv_ps", bufs=1)
            for c in range(2):
                nc.tensor.matmul(
                    out=k_ps,
                    lhsT=wqkvT[c][:, C + 128 * g:C + 128 * (g + 1)],
                    rhs=h_sb[c][:, 512 * j:512 * (j + 1)],
                    start=(c == 0), stop=(c == 1))
            nc.vector.tensor_scalar(out=k_sb[g][:, 512 * j:512 * (j + 1)],
                                    in0=k_ps,
                                    scalar1=qkb[:, 2 + g:3 + g], scalar2=None,
                                    op0=mybir.AluOpType.add)
            dr_shuffle(k8[g], k_sb[g], 512, 512 * j)

        def v_chunk(t, pool):
            kv = pool.tile([128, 512], FP32, name="kv_ps", tag="kv_ps", bufs=1)
            v_ps = kv[:, 0:NH * HD]
            for c in range(2):
                nc.tensor.matmul(
                    out=v_ps,
                    lhsT=h_sb[c][:, 128 * t:128 * (t + 1)],
                    rhs=wqkvT[c][:, 2 * C:3 * C],
                    start=(c == 0), stop=(c == 1))
            nc.vector.tensor_copy(
                out=vT[:, t, :, 0:HD],
                in_=v_ps.rearrange("p (a b) -> p a b", a=NH))

        # ===== P1: GroupNorm (streamed) + start of qkv projections ========
        with tc.tile_pool(name="gn", bufs=1) as gn_pool, \
             tc.tile_pool(name="psum_small", bufs=1, space="PSUM") as psum_small:
            x_sb = [gn_pool.tile([128, S], FP32, name=f"x{c}", tag=f"x{c}") for c in range(2)]
            # slab-wise DMA so bn_stats can start before the full tile lands;
            # alternate two DMA queues so the 16 slabs stream in parallel
            for c in range(2):
                for i in range(8):
                    q = nc.sync if (i % 2 == 0) else nc.gpsimd
                    q.dma_start(
                        out=x_sb[c][:, 512 * i:512 * (i + 1)],
                        in_=x_d[128 * c:128 * (c + 1), 512 * i:512 * (i + 1)])

            # per-channel stats via bn_stats/bn_aggr (free dim), then 8-channel
            # group combine via a tiny matmul against the group-indicator matrix.
            msq = [gn_pool.tile([128, 3], FP32, name=f"msq{c}", tag=f"msq{c}") for c in range(2)]
            gstat_ps = psum_small.tile([32, 3], FP32, name="gstat_ps", tag="gstat_ps")
            for c in range(2):
                xv = x_sb[c].rearrange("p (a b) -> p a b", b=512)
                stats = gn_pool.tile([128, 8, 6], FP32, name=f"stats{c}", tag=f"stats{c}")
                for i in range(8):
                    nc.vector.bn_stats(out=stats[:, i, :], in_=xv[:, i, :])
                mv = gn_pool.tile([128, 2], FP32, name=f"mv{c}", tag=f"mv{c}")
                nc.vector.bn_aggr(out=mv, in_=stats)
                # msq = [mean, var, mean^2]
                nc.vector.tensor_copy(out=msq[c][:, 0:2], in_=mv)
                nc.vector.tensor_mul(out=msq[c][:, 2:3], in0=mv[:, 0:1], in1=mv[:, 0:1])
            for c in range(2):
                nc.tensor.matmul(out=gstat_ps, lhsT=gmat[:, 32 * c:32 * (c + 1)],
                                 rhs=msq[c], start=(c == 0), stop=(c == 1))
            # grp_mean = s0, grp_var = s1 + s2 - s0^2
            gstat = gn_pool.tile([32, 3], FP32, name="gstat", tag="gstat")
            nc.vector.tensor_copy(out=gstat, in_=gstat_ps)
            gvar = gn_pool.tile([32, 1], FP32, name="gvar", tag="gvar")
            gm2 = gn_pool.tile([32, 1], FP32, name="gm2", tag="gm2")
            nc.vector.tensor_mul(out=gm2, in0=gstat[:, 0:1], in1=gstat[:, 0:1])
            nc.vector.tensor_add(out=gvar, in0=gstat[:, 1:2], in1=gstat[:, 2:3])
            nc.vector.tensor_tensor(out=gvar, in0=gvar, in1=gm2,
                                    op=mybir.AluOpType.subtract)
            # rstd = 1/sqrt(var+eps) = exp(-0.5*ln(var+eps)); Ln+Exp share
            # one ACT table set with the attention exps (no table switch)
            nc.scalar.activation(out=gvar, in_=gvar,
                                 func=mybir.ActivationFunctionType.Ln,
                                 bias=eps_t, scale=1.0)
            nc.scalar.activation(out=gvar, in_=gvar,
                                 func=mybir.ActivationFunctionType.Exp,
                                 bias=0.0, scale=-0.5)

            # one broadcast DMA per chunk for [mean, rstd], then per-channel
            # affine A = rstd*gamma, B = beta - mean*A
            mr = gn_pool.tile([32, 2], FP32, name="mr", tag="mr")
            nc.vector.tensor_copy(out=mr[:, 0:1], in_=gstat[:, 0:1])
            nc.vector.tensor_copy(out=mr[:, 1:2], in_=gvar)
            Ab, Bb = [], []
            for c in range(2):
                mr_rep = gn_pool.tile([128, 2], FP32, name=f"mr_rep{c}", tag=f"mr_rep{c}")
                src = mr[16 * c:16 * (c + 1), :]
                bc = bass.AP(tensor=src.tensor, offset=src.offset,
                             ap=[src.ap[0], [0, 8], src.ap[1]])
                nc.sync.dma_start(out=mr_rep, in_=bc)
                A = gn_pool.tile([128, 1], FP32, name=f"A{c}", tag=f"A{c}")
                Bt = gn_pool.tile([128, 1], FP32, name=f"B{c}", tag=f"B{c}")
                nc.vector.tensor_mul(out=A, in0=mr_rep[:, 1:2], in1=gamma[c])
                nc.vector.tensor_mul(out=Bt, in0=mr_rep[:, 0:1], in1=A)
                nc.vector.tensor_tensor(out=Bt, in0=beta[c], in1=Bt,
                                        op=mybir.AluOpType.subtract)
                Ab.append(A)
                Bb.append(Bt)

            # apply h in 512-col slabs; q and the first k chunks stream in as
            # soon as their h columns exist, so the PE starts while the DVE
            # is still applying later slabs
            with tc.tile_pool(name="p2psum", bufs=1, space="PSUM") as p2:
                for j in range(8):
                    for c in range(2):
                        sl = slice(512 * j, 512 * (j + 1))
                        nc.vector.tensor_scalar(out=h_sb[c][:, sl],
                                                in0=x_sb[c][:, sl],
                                                scalar1=Ab[c], scalar2=Bb[c],
                                                op0=mybir.AluOpType.mult,
                                                op1=mybir.AluOpType.add)
                    if j <= 1:
                        k_chunk(0, j, p2)
                    if j == 1:
                        for g in range(2):
                            q_ps = p2.tile([128, QBLK], FP32, name="q_ps",
                                           tag="q_ps", bufs=1)
                            for n in range(2):
                                for c in range(2):
                                    nc.tensor.matmul(
                                        out=q_ps[:, 512 * n:512 * (n + 1)],
                                        lhsT=wqkvT[c][:, 128 * g:128 * (g + 1)],
                                        rhs=h_sb[c][:, 512 * n:512 * (n + 1)],
                                        start=(c == 0), stop=(c == 1))
                            nc.vector.tensor_scalar(out=q_sb[g], in0=q_ps,
                                                    scalar1=qkb[:, g:g + 1],
                                                    scalar2=None,
                                                    op0=mybir.AluOpType.add)
                            dr_shuffle(q8[g], q_sb[g], QBLK, 0)
                nc.vector.tensor_copy(
                    out=vT[:, :, :, HD:HD + 1],
                    in_=ones_f[:, 0:256].rearrange("p (a b c) -> p a b c", b=NH, c=1))

        # ================= P3: attention (head pairs) =====================
        with tc.tile_pool(name="att_psum", bufs=1, space="PSUM") as att_ps, \
             tc.tile_pool(name="acc_psum", bufs=1, space="PSUM") as acc_ps, \
             tc.tile_pool(name="kv_psum", bufs=1, space="PSUM") as kv_ps, \
             tc.tile_pool(name="exp_pool", bufs=6) as exp_pool, \
             tc.tile_pool(name="sums_pool", bufs=3) as sums_pool:
            for p in range(4):
                g, lh = p // 2, 2 * (p % 2)
                for w in range(2):
                    # both heads' AV accumulators packed into ONE psum bank
                    # (head hh at partitions 64*hh); bf16 AV operands accept
                    # the base-64 output partition (fp32r does not)
                    acc = acc_ps.tile([128, 512], FP32, name="acc", tag="acc")
                    # depth-3 logits pipeline, AV lagged 2 behind QK: fp8
                    # DoubleRow QK makes the PE faster per iteration than
                    # the ACT exp stream, so with enough buffering the PE
                    # runs ahead, every exp/AV wait is pre-satisfied, and
                    # no semaphore wake latency lands on the loop period
                    lg = [att_ps.tile([128, 1024], FP32, name=f"lg{i}",
                                      tag=f"lg{i}") for i in range(3)]
                    expq = {}
                    for t in range(34):
                        if t < 32:
                            if p == 0 and w == 0:
                                v_chunk(t, kv_ps)
                                if t % 4 == 0 and t // 4 + 2 <= 7:
                                    k_chunk(0, t // 4 + 2, kv_ps)
                            if p == 1 and t % 8 == 0:
                                k_chunk(1, 4 * w + t // 8, kv_ps)
                            buf = lg[t % 3]
                            for hh in range(2):
                                h = lh + hh
                                nc.tensor.matmul(
                                    out=buf[:, 512 * hh:512 * (hh + 1)],
                                    lhsT=k8[g][32 * h:32 * h + 16, :, 128 * t:128 * (t + 1)],
                                    rhs=q8[g][32 * h:32 * h + 16, :, 512 * w:512 * (w + 1)],
                                    start=True, stop=True,
                                    perf_mode=mybir.MatmulPerfMode.DoubleRow,
                                    tile_position=(32 * h, 0))
                        if t >= 2:
                            eT = expq.pop(t - 2)
                            for hh in range(2):
                                nc.tensor.matmul(
                                    out=acc[64 * hh:64 * hh + HD + 1, :],
                                    lhsT=vT[:, t - 2, 4 * g + lh + hh, 0:HD + 1],
                                    rhs=eT[:, 512 * hh:512 * (hh + 1)],
                                    start=(t - 2 == 0), stop=(t - 2 == 31),
                                    tile_position=(0, 64 * hh))
                        if t < 32:
                            expT = exp_pool.tile([128, 1024], BF16, name="expT",
                                                 tag="expT")
                            nc.scalar.activation(out=expT, in_=buf,
                                                 func=mybir.ActivationFunctionType.Exp,
                                                 scale=ATT_SCALE)
                            expq[t] = expT
                    # DVE lanes cannot shift partitions: stage each head at
                    # its acc partition base, then move with SBUF->SBUF DMA.
                    st = sums_pool.tile([128, 512], FP32, name="st", tag="st")
                    for hh in range(2):
                        h = lh + hh
                        base = 64 * hh
                        nc.vector.tensor_copy(
                            out=st[base:base + HD + 1, :],
                            in_=acc[base:base + HD + 1, :])
                        nc.sync.dma_start(
                            out=o0[g][32 * h:32 * (h + 1), 512 * w:512 * (w + 1)],
                            in_=st[base:base + HD, :])
                        # reciprocal in place, then straight to DRAM for the
                        # later broadcast.
                        nc.vector.reciprocal(out=st[base + HD:base + HD + 1, :],
                                             in_=st[base + HD:base + HD + 1, :])
                        nc.gpsimd.dma_start(
                            out=recipS_d[4 * g + h, 512 * w:512 * (w + 1)],
                            in_=st[base + HD:base + HD + 1, :])
                if p % 2 == 1 and w == 1:
                    # this head-group's o0 is complete: normalize it now so
                    # only the projection remains after the last exp
                    src_r = recipS_d[4 * g:4 * (g + 1), :]
                    bc = bass.AP(tensor=src_r.tensor, offset=src_r.offset,
                                 ap=[src_r.ap[0], [0, 32], src_r.ap[1]])
                    nc.sync.dma_start(out=reps[g], in_=bc)
                    nc.vector.tensor_mul(out=o0n[g], in0=o0[g], in1=reps[g])

        # ================= P4: proj + bias + residual =====================
        with tc.tile_pool(name="norm", bufs=1) as norm_pool:
            xres = [norm_pool.tile([128, QBLK], FP32, name=f"xres{m}", tag=f"xres{m}") for m in range(2)]
            for m in range(2):
                nc.sync.dma_start(out=xres[m], in_=x_d[128 * m:128 * (m + 1), 0:QBLK])
            with tc.tile_pool(name="p4psum", bufs=2, space="PSUM") as p4:
                for m in range(2):
                    o_ps = p4.tile([128, QBLK], FP32, name="o_ps", tag="o_ps")
                    for n in range(2):
                        sl = slice(512 * n, 512 * (n + 1))
                        for c in range(2):
                            nc.tensor.matmul(
                                out=o_ps[:, sl],
                                lhsT=wprojT[c][:, 128 * m:128 * (m + 1)],
                                rhs=o0n[c][:, sl],
                                start=(c == 0), stop=False)
                        nc.tensor.matmul(
                            out=o_ps[:, sl],
                            lhsT=pb_row[:, 128 * m:128 * (m + 1)],
                            rhs=ones_q[:, sl],
                            start=False, stop=True)
                    out_sb = norm_pool.tile([128, QBLK], FP32, name=f"outsb{m}", tag=f"outsb{m}")
                    nc.vector.tensor_add(out=out_sb, in0=o_ps, in1=xres[m])
                    nc.sync.dma_start(out=out_d[128 * m:128 * (m + 1), :], in_=out_sb)

            if dbg_d is not None:
                f32 = lambda ap: ap.bitcast(FP32)
                nc.sync.dma_start(out=dbg_d["h0"], in_=f32(h_sb[0]))
                nc.sync.dma_start(out=dbg_d["h1"], in_=f32(h_sb[1]))
                nc.sync.dma_start(out=dbg_d["q0"], in_=f32(q_sb[0]))
                nc.sync.dma_start(out=dbg_d["k0"], in_=f32(k_sb[0]))
                nc.sync.dma_start(out=dbg_d["vT"],
                                  in_=f32(vT.rearrange("p a b c -> p (a b c)")))
                nc.sync.dma_start(out=dbg_d["o00"], in_=o0[0])
                nc.sync.dma_start(out=dbg_d["o01"], in_=o0[1])
                nc.sync.dma_start(out=dbg_d["recipS"], in_=recipS_d)
                nc.sync.dma_start(out=dbg_d["rep0"], in_=reps[0])
                nc.sync.dma_start(out=dbg_d["o0n0"], in_=f32(o0n[0]))


def _host_inputs(x, gn_gamma, gn_beta, qkv_w, qkv_b, proj_w, proj_b):
    B_, C_, D, H, W = x.shape
    S_ = D * H * W
    assert (C_, S_) == (C, S) and B_ == 2
    xf = np.ascontiguousarray(np.asarray(x, np.float32).reshape(B_, C_, S_))
    wqkvT = np.ascontiguousarray(np.asarray(qkv_w, np.float32).T)
    qkb = np.ascontiguousarray(np.asarray(qkv_b[:512], np.float32).reshape(4, 128).T)
    wprojT = np.ascontiguousarray(np.asarray(proj_w, np.float32).T)
    # v-bias folds into the projection bias: proj(o/S + vb) = proj(o/S) + Wp@vb
    pb_eff = np.asarray(proj_b, np.float64) + \
        np.asarray(proj_w, np.float64) @ np.asarray(qkv_b[512:], np.float64)
    pb_row = np.ascontiguousarray(pb_eff.astype(np.float32).reshape(1, C))
    gamma = np.ascontiguousarray(np.asarray(gn_gamma, np.float32).reshape(C, 1))
    beta = np.ascontiguousarray(np.asarray(gn_beta, np.float32).reshape(C, 1))
    gmat = np.zeros((128, 64), np.float32)
    for c in range(2):
        for p in range(128):
            gmat[p, 32 * c + (128 * c + p) // 8] = 0.125
    in_maps = []
    for core in range(NCORES):
        b, qb = core // 4, core % 4
        off = qb * QBLK
        xrot = np.concatenate([xf[b][:, off:], xf[b][:, :off]], axis=1)
        in_maps.append(dict(
            x=np.ascontiguousarray(xrot), wqkvT=wqkvT, qkb=qkb,
            wprojT=wprojT, pb_row=pb_row, gamma=gamma, beta=beta, gmat=gmat))
    return in_maps


_NC_CACHE = None
_EXEC_CACHE = None


class _Exec:
    """Cached PJRT executable for a compiled Bass module.

    run_bass_kernel_spmd (axon path -> run_bass_via_pjrt) builds a fresh
    jax.jit closure per call, re-tracing and re-compiling the XLA wrapper
    every time. Build the sharded executable once and reuse it; inputs are
    re-uploaded only when their bytes change (crc32+adler32 fingerprint).
    """

    def __init__(self, nc, n_cores):
        import jax
        from jax.sharding import Mesh, NamedSharding, PartitionSpec
        from jax.experimental.shard_map import shard_map
        from concourse.bass2jax import (_bass_exec_p, install_neuronx_cc_hook,
                                        partition_id_tensor)
        install_neuronx_cc_hook()
        self.jax = jax
        self.n_cores = n_cores
        pname = nc.partition_id_tensor.name if nc.partition_id_tensor else None
        in_names, out_names, out_avals, zero_outs = [], [], [], []
        for alloc in nc.m.functions[0].allocations:
            if not isinstance(alloc, mybir.MemoryLocationSet):
                continue
            name = alloc.memorylocations[0].name
            if alloc.kind == "ExternalInput":
                if name != pname:
                    in_names.append(name)
            elif alloc.kind == "ExternalOutput":
                out_names.append(name)
                shape = tuple(alloc.tensor_shape)
                dtype = mybir.dt.np(alloc.dtype)
                out_avals.append(jax.core.ShapedArray(shape, dtype))
                zero_outs.append(np.zeros(shape, dtype))
        self.in_names, self.out_names = in_names, out_names
        self.out_avals = out_avals
        all_in = list(in_names) + list(out_names) + ([pname] if pname else [])

        def _body(*args):
            operands = list(args)
            if pname is not None:
                operands.append(partition_id_tensor())
            return tuple(_bass_exec_p.bind(
                *operands, out_avals=tuple(out_avals), in_names=tuple(all_in),
                out_names=tuple(out_names), lowering_input_output_aliases=(),
                sim_require_finite=True, sim_require_nnan=True, nc=nc))

        devices = jax.devices()[:n_cores]
        mesh = Mesh(np.asarray(devices), ("core",))
        specs = (PartitionSpec("core"),) * (len(in_names) + len(out_names))
        self.fn = jax.jit(
            shard_map(_body, mesh=mesh, in_specs=specs,
                      out_specs=(PartitionSpec("core"),) * len(out_names),
                      check_rep=False), keep_unused=True)
        self.sh = NamedSharding(mesh, PartitionSpec("core"))
        self.dev_zero = [jax.device_put(
            np.zeros((n_cores * z.shape[0], *z.shape[1:]), z.dtype), self.sh)
            for z in zero_outs]
        self._in_cache = {}

    def device_inputs(self, in_maps):
        import zlib
        dev_in = []
        for nm in self.in_names:
            glob = np.ascontiguousarray(np.concatenate(
                [np.asarray(in_maps[c][nm]) for c in range(self.n_cores)], 0))
            mv = memoryview(glob).cast("B")
            fp = (zlib.crc32(mv), zlib.adler32(mv), glob.shape, glob.dtype.str)
            hit = self._in_cache.get(nm)
            if hit is None or hit[0] != fp:
                hit = (fp, self.jax.device_put(glob, self.sh))
                self._in_cache[nm] = hit
            dev_in.append(hit[1])
        return dev_in

    def run(self, in_maps):
        outs = self.fn(*self.device_inputs(in_maps), *self.dev_zero)
        outs = [np.asarray(o).reshape(self.n_cores, *self.out_avals[i].shape)
                for i, o in enumerate(outs)]
        return [{nm: outs[i][c] for i, nm in enumerate(self.out_names)}
                for c in range(self.n_cores)]


def kernel(x, gn_gamma, gn_beta, qkv_w, qkv_b, proj_w, proj_b):
    global _NC_CACHE, _EXEC_CACHE
    in_maps = _host_inputs(x, gn_gamma, gn_beta, qkv_w, qkv_b, proj_w, proj_b)
    if _NC_CACHE is None:
        _NC_CACHE = build_nc()
        res = run_bass_kernel_spmd(_NC_CACHE, in_maps,
                                   core_ids=list(range(NCORES)))
        results = res.results
    else:
        if _EXEC_CACHE is None:
            _EXEC_CACHE = _Exec(_NC_CACHE, NCORES)
        results = _EXEC_CACHE.run(in_maps)
    B_, C_, D, H, W = x.shape
    full = np.empty((B_, C, S), np.float32)
    for core in range(NCORES):
        b, qb = core // 4, core % 4
        full[b][:, qb * QBLK:(qb + 1) * QBLK] = results[core]["out"]
    return full.reshape(B_, C, D, H, W)

